# revision 5
# baseline (speedup 1.0000x reference)
"""Trainium2 Bass kernel for EntityMarker segment-reduce (span means).

Problem: sequence_output [128, 2048, 768] f32, entity_positions [128, 4] int.
For each batch b, compute the mean of sequence_output[b, s:e+1, :] for the
head span (cols 0,1) and tail span (cols 2,3), clamped like the reference.
Output: (head [128, 768], tail [128, 768]) f32.

Strategy (data-parallel over batch, 8 cores, load-balanced):
  - On host, compute clamped spans; per batch the union of the two spans is
    1-2 contiguous runs of rows. Only those rows (~26% of the tensor) are
    read on device. Batches are assigned to cores by greedy LPT on union
    size (16 batches/core) to balance per-core bytes.
  - Each run is covered by K-row windows. A gpsimd indirect DMA
    (InstDMACopy + dynamic offset) gathers one window per partition:
    out[p, :] = x[start[p] : start[p]+K] - 128 windows / instruction,
    48KB-contiguous HBM reads per descriptor.
  - Per window column j, a PE matmul accumulates W_j[128, 32]^T @ rows into
    PSUM [32, 768]: 32 segment means (16 head + 16 tail) per core. Weights
    are 1/span_len for rows inside the span and 0 for overhang/padding, so
    PSUM holds the means directly.
  - The device program is UNIFORM across cores (SPMD); all data-dependence
    is carried via input tensors (x shard, window starts, weights).
"""

import numpy as np

_B, _L, _H = 128, 2048, 768
_NCORES = 8
_BPC = _B // _NCORES  # batches per core
_SEG = 2 * _BPC       # segments per core: 16 head + 16 tail
_K = 8                # rows per window (per-partition contiguous read)

_prog_cache = {}


def _build_program(n_inst):
    import concourse.bass as bass
    import concourse.mybir as mybir
    from concourse import bacc, tile

    f32 = mybir.dt.float32
    i32 = mybir.dt.int32
    n_mm = n_inst * _K  # 128-row matmul chunks

    nc = bacc.Bacc(None, target_bir_lowering=False)
    x = nc.declare_dram_parameter("x", [_BPC * _L, _H], f32, isOutput=False)
    idx = nc.declare_dram_parameter("idx", [128, n_inst], i32, isOutput=False)
    w = nc.declare_dram_parameter("w", [128, n_mm * _SEG], f32, isOutput=False)
    out = nc.declare_dram_parameter("out", [_SEG, _H], f32, isOutput=True)

    with tile.TileContext(nc) as tc:
        with (
            tc.tile_pool(name="const", bufs=1) as cpool,
            tc.tile_pool(name="gather", bufs=4) as gpool,
            tc.tile_pool(name="psum", bufs=1, space="PSUM") as ppool,
        ):
            idx_t = cpool.tile([128, n_inst], i32)
            nc.sync.dma_start(out=idx_t[:], in_=idx[:])
            w_t = cpool.tile([128, n_mm * _SEG], f32)
            nc.sync.dma_start(out=w_t[:], in_=w[:])

            ps_a = ppool.tile([_SEG, 512], f32)
            ps_b = ppool.tile([_SEG, 256], f32)

            for t in range(n_inst):
                # NOTE: the gather out AP must be 2D — a 3D [128, K, H]
                # AP mis-gathers on HW (sim doesn't model it).
                g = gpool.tile([128, _K * _H], f32, tag="g")
                nc.gpsimd.indirect_dma_start(
                    out=g[:],
                    out_offset=None,
                    in_=x[:],
                    in_offset=bass.IndirectOffsetOnAxis(
                        ap=idx_t[:, t:t + 1], axis=0),
                )
                for j in range(_K):
                    c = t * _K + j
                    lhsT = w_t[:, c * _SEG:(c + 1) * _SEG]
                    nc.tensor.matmul(
                        ps_a[:], lhsT, g[:, j * _H:j * _H + 512],
                        start=(c == 0), stop=(c == n_mm - 1),
                    )
                    nc.tensor.matmul(
                        ps_b[:], lhsT, g[:, j * _H + 512:(j + 1) * _H],
                        start=(c == 0), stop=(c == n_mm - 1),
                    )

            o_t = cpool.tile([_SEG, _H], f32)
            nc.vector.tensor_copy(o_t[:, 0:512], ps_a[:])
            nc.vector.tensor_copy(o_t[:, 512:768], ps_b[:])
            nc.sync.dma_start(out=out[:], in_=o_t[:])
    nc.compile()
    return nc


def _spans(entity_positions):
    ep = np.asarray(entity_positions).astype(np.int64)
    hs = np.clip(ep[:, 0], 0, _L - 1)
    he = np.maximum(hs, np.minimum(ep[:, 1], _L - 1))
    ts = np.clip(ep[:, 2], 0, _L - 1)
    te = np.maximum(ts, np.minimum(ep[:, 3], _L - 1))
    return hs, he, ts, te


def _plan(entity_positions):
    """Per-core batch assignment, window starts and weight rows."""
    hs, he, ts, te = _spans(entity_positions)

    runs = []
    usize = np.zeros(_B, np.int64)
    for b in range(_B):
        a0, a1, b0, b1 = hs[b], he[b], ts[b], te[b]
        if a0 > b0:
            a0, a1, b0, b1 = b0, b1, a0, a1
        if b0 <= a1 + 1:
            r = [(int(a0), int(max(a1, b1)))]
        else:
            r = [(int(a0), int(a1)), (int(b0), int(b1))]
        runs.append(r)
        usize[b] = sum(e - s + 1 for s, e in r)

    # greedy LPT assignment: heaviest batches first to the lightest core
    order = np.argsort(-usize, kind="stable")
    loads = np.zeros(_NCORES, np.int64)
    core_batches = [[] for _ in range(_NCORES)]
    for b in order:
        open_cores = [c for c in range(_NCORES) if len(core_batches[c]) < _BPC]
        c = min(open_cores, key=lambda i: loads[i])
        core_batches[c].append(int(b))
        loads[c] += usize[b]

    max_start = _BPC * _L - _K
    starts = [[] for _ in range(_NCORES)]   # window start rows
    wrows = [[] for _ in range(_NCORES)]    # per-window [K, SEG] weights
    for c in range(_NCORES):
        for lb, b in enumerate(core_batches[c]):
            base = lb * _L
            hw_ = np.float32(1.0 / (he[b] - hs[b] + 1))
            tw_ = np.float32(1.0 / (te[b] - ts[b] + 1))
            for (s, e) in runs[b]:
                t = s
                while t <= e:
                    wstart = min(base + t, max_start)
                    hi = min(e, wstart - base + _K - 1)
                    wr = np.zeros((_K, _SEG), np.float32)
                    r = wstart - base + np.arange(_K)
                    new = (r >= t) & (r <= hi)
                    wr[new & (r >= hs[b]) & (r <= he[b]), lb] = hw_
                    wr[new & (r >= ts[b]) & (r <= te[b]), _BPC + lb] = tw_
                    starts[c].append(wstart)
                    wrows[c].append(wr)
                    t = hi + 1

    n_win = max(len(s) for s in starts)
    n_inst = (n_win + 127) // 128
    n_win = n_inst * 128

    idx_mats, w_mats = [], []
    for c in range(_NCORES):
        pad = n_win - len(starts[c])
        st = np.array(starts[c] + [0] * pad, np.int32)
        wr = np.stack(
            wrows[c] + [np.zeros((_K, _SEG), np.float32)] * pad
        )  # [n_win, K, SEG]
        # window W_i -> instruction i = W_i // 128, partition p = W_i % 128
        idx_mat = np.ascontiguousarray(st.reshape(n_inst, 128).T)  # [128,n_inst]
        # w[p, ((i*K)+j)*SEG + m] = wr[i*128 + p, j, m]
        w_mat = np.ascontiguousarray(
            wr.reshape(n_inst, 128, _K, _SEG)
            .transpose(1, 0, 2, 3)
            .reshape(128, n_inst * _K * _SEG)
        )
        idx_mats.append(idx_mat)
        w_mats.append(w_mat)

    return core_batches, idx_mats, w_mats, n_inst


def _run(sequence_output, entity_positions, trace=False, trace_cores=None):
    from concourse.bass_utils import run_bass_kernel_spmd

    x = np.ascontiguousarray(np.asarray(sequence_output), dtype=np.float32)
    core_batches, idx_mats, w_mats, n_inst = _plan(entity_positions)

    if n_inst not in _prog_cache:
        _prog_cache[n_inst] = _build_program(n_inst)
    nc = _prog_cache[n_inst]

    in_maps = []
    for c in range(_NCORES):
        xc = np.ascontiguousarray(x[core_batches[c]]).reshape(_BPC * _L, _H)
        in_maps.append({"x": xc, "idx": idx_mats[c], "w": w_mats[c]})

    res = run_bass_kernel_spmd(
        nc, in_maps, list(range(_NCORES)), trace=trace,
        trace_cores=trace_cores,
    )

    head = np.zeros((_B, _H), np.float32)
    tail = np.zeros((_B, _H), np.float32)
    for c in range(_NCORES):
        o = res.results[c]["out"]
        for lb, b in enumerate(core_batches[c]):
            head[b] = o[lb]
            tail[b] = o[_BPC + lb]
    return (head, tail), res


def kernel(sequence_output, entity_positions):
    (head, tail), _ = _run(sequence_output, entity_positions)
    return head, tail
